# revision 17
# baseline (speedup 1.0000x reference)
"""Bahdanau-style attention kernel for Trainium2 (8 NeuronCores, batch-parallel).

Computes, for B=16, S=4096, H=512:
    hid  = hidden @ W_attn[:H] + b_attn                       (B, H)
    en   = tanh(hid[:,None,:] + enc @ W_attn[H:])             (B, S, H)
    lg   = en @ v                                             (B, S, 1)
    w    = softmax(lg, axis=1)
    ctx  = w^T @ enc                                          (B, 1, 2H)

Sharding: data-parallel over batch, 2 batches per core. Per core:
  - enc streams from HBM twice: e-major f16 full rows (whole-batch SBUF
    residency, 8KB descriptors, feeds the context accumulation) and fp8
    e4m3 with e-tile pairs element-interleaved (group-streamed, feeds
    the double-pumped DoubleRow projection matmul, 2 e-tiles/matmul).
    We is pre-scaled by SCALE_W to dodge the e4m3 subnormal range; the
    1/SCALE_W is folded into tanh's scale.
  - hid projection is precomputed host-side (it is O(B*H*H), trivial
    next to the O(B*S*E*H) device work) and enters as tanh's bias.
  - logits use v replicated across 128 partitions as the f16 stationary
    so exp(logits) lands pre-broadcast; exp emits f16 weights (the
    constant shift keeps every weight inside f16 normal range) and its
    accum_out yields the softmax normalizer.
  - context = sum_s w_s * enc16[e, s]: mostly fused scalar_tensor_tensor
    on DVE; CTX_TT e-tiles per group instead run DVE 2x tensor_tensor
    with the accumulation on the scalar engine (load balancing).
No cross-core communication; output gathered on host.
"""

import os
import numpy as np
import ml_dtypes
from contextlib import ExitStack

import concourse.bacc as bacc
import concourse.tile as tile
from concourse import mybir
from concourse.bass_utils import run_bass_kernel_spmd

F32 = mybir.dt.float32
F16 = mybir.dt.float16
F8 = mybir.dt.float8e4

B, S, H = 16, 4096, 512
E = 2 * H                      # 1024 encoder feature dim
NCORES = 8
BPC = B // NCORES              # batches per core = 2
ET = E // 128                  # 8 e-tiles
HT = H // 128                  # 4 h-tiles
NPAIR = ET // 2                # 4 e-tile pairs for DoubleRow
SBLK = 512                     # s-block width
NSB = S // SBLK                # 8 s-blocks per batch
GROUP_SB = 2
GW = GROUP_SB * SBLK

SCALE_W = 32.0                 # We pre-scale for e4m3 range
SHIFT = 5.0                    # exp(logit - SHIFT): logits empirically in [-3.4, 3.4]

NF8 = int(os.environ.get("ATTN_NF8", "4"))
NWARM = int(os.environ.get("ATTN_NWARM", "18"))
CTX_TT = int(os.environ.get("ATTN_CTX_TT", "2"))

GROUPS = [[0, 1], [2, 3], [4, 5], [6], [7]]
NGRP = len(GROUPS)

TRACE = False          # set by test harness; harness-default off
LAST_RESULTS = None    # last BassKernelResults (for profiling in test.py)

_NC_CACHE = {}


def _build():
    nc = bacc.Bacc("TRN2", target_bir_lowering=False, debug=False)

    CW = HT * BPC + 1          # hid columns | -SHIFT
    encT = nc.dram_tensor("encT", [BPC, E, S], F16, kind="ExternalInput").ap()
    encT8 = nc.dram_tensor("encT8", [BPC, NPAIR * 128, S * 2], F8,
                           kind="ExternalInput").ap()
    We8_d = nc.dram_tensor("We8", [128, ET * H], F8, kind="ExternalInput").ap()
    We16_d = nc.dram_tensor("We16", [128, ET * H], F16, kind="ExternalInput").ap()
    V_d = nc.dram_tensor("V128", [128, HT * 128], F16, kind="ExternalInput").ap()
    cst_d = nc.dram_tensor("consts", [128, CW], F32, kind="ExternalInput").ap()
    ctx_d = nc.dram_tensor("ctx", [BPC, E], F32, kind="ExternalOutput").ap()

    with tile.TileContext(nc) as tc, ExitStack() as ctx:
        cpool = ctx.enter_context(tc.tile_pool(name="consts", bufs=1))
        fpool = ctx.enter_context(tc.tile_pool(name="encb", bufs=2))
        e8pool = ctx.enter_context(tc.tile_pool(name="enc8", bufs=4))
        tpool = ctx.enter_context(tc.tile_pool(name="tanh", bufs=2))
        wpool = ctx.enter_context(tc.tile_pool(name="wexp", bufs=3))
        jpool = ctx.enter_context(tc.tile_pool(name="junk", bufs=2))
        dpool = ctx.enter_context(tc.tile_pool(name="defer", bufs=5))
        spool = ctx.enter_context(tc.tile_pool(name="stats", bufs=1))
        pp = ctx.enter_context(tc.tile_pool(name="pproj", bufs=2, space="PSUM"))
        pl = ctx.enter_context(tc.tile_pool(name="plog", bufs=2, space="PSUM"))

        # ---- PE warm-up: dummy matmuls while DMAs land (HAM -> K=8/8) ----
        wlhs = cpool.tile([128, 128], F16)
        wrhs = cpool.tile([128, 256], F16)
        nc.vector.memset(wlhs[:], 0.0)
        nc.vector.memset(wrhs[:], 0.0)
        wps = pl.tile([128, GW], F32, name="lg", tag="lg")
        for _ in range(NWARM):
            nc.tensor.matmul(wps[:, 0:256], wlhs[:], wrhs[:], start=True, stop=True)

        # ---- constants ----
        cst_sb = cpool.tile([128, CW], F32)
        nc.scalar.dma_start(cst_sb[:], cst_d)
        We8_sb = cpool.tile([128, ET * H], F8)
        nc.scalar.dma_start(We8_sb[:], We8_d)
        V_sb = cpool.tile([128, HT * 128], F16)
        nc.scalar.dma_start(V_sb[:], V_d)
        We16_sb = None
        if NF8 < 4:
            We16_sb = cpool.tile([128, ET * H], F16)
            nc.scalar.dma_start(We16_sb[:], We16_d)
        hid_sb = cst_sb[:, 0:HT * BPC]            # bias col per (h, b)
        shift_col = cst_sb[:, HT * BPC:HT * BPC + 1]

        # ---- stats accumulators ----
        zslots = spool.tile([128, BPC * NGRP], F32)
        cslots = spool.tile([128, BPC * ET * NGRP], F32)
        ctx_red = spool.tile([128, BPC * ET], F32)
        zred = spool.tile([128, BPC], F32)
        zrec = spool.tile([128, BPC], F32)
        ctx_fin = spool.tile([128, BPC * ET], F32)

        out_dmas = []
        encbs = {}

        def emit_rows(bb, s0, s1):
            # f16 partial rows (2-4KB runs) for batch bb on the sync queue
            for t in range(ET):
                nc.sync.dma_start(
                    encbs[bb][:, t * S + s0:t * S + s1],
                    encT[bb].rearrange("(t p) s -> p t s", p=128)[
                        :, t, s0:s1],
                )

        for b in range(BPC):
            pending = []          # (tile, col, gw) scalar accums, one group late
            # ---- DMA emission pass: trigger order on the sync queue is
            # fp8 for groups 0..3, then the batch's f16 full rows, then
            # g4's fp8 (its pool-WAR semaphore delays it anyway) ----
            encg8s = []

            def emit_fp8(g, sbs):
                gsb = len(sbs)
                gw = gsb * SBLK
                g0 = sbs[0]
                encg8 = e8pool.tile([128, ET * GW], F8, name="encg8",
                                    tag="encg8")
                if b == 0 and g == 0:
                    # first data on the wire: pair 0, first s-block only
                    nc.sync.dma_start(
                        encg8[:, 0:2 * SBLK],
                        encT8[0].rearrange("(j p) s2 -> p j s2", p=128)[
                            :, 0, 0:2 * SBLK])
                    nc.sync.dma_start(
                        encg8[:, 2 * SBLK:2 * gw],
                        encT8[0].rearrange("(j p) s2 -> p j s2", p=128)[
                            :, 0, 2 * SBLK:2 * gw])
                    jchunks = [(1, 1), (2, 2)]
                else:
                    jchunks = [(0, 2), (2, 2)]
                for j0, nj in jchunks:
                    nc.sync.dma_start(
                        encg8[:, j0 * 2 * gw:(j0 + nj) * 2 * gw].rearrange(
                            "p (j s2) -> p j s2", j=nj),
                        encT8[b].rearrange("(j p) s2 -> p j s2", p=128)[
                            :, j0:j0 + nj,
                            g0 * SBLK * 2:g0 * SBLK * 2 + 2 * gw],
                    )
                encg8s.append(encg8)

            if b not in encbs:
                encbs[b] = fpool.tile([128, ET * S], F16, name="encb",
                                      tag="encb")
            emit_fp8(0, GROUPS[0])
            emit_fp8(1, GROUPS[1])
            emit_rows(b, 0, 2048)
            emit_fp8(2, GROUPS[2])
            emit_fp8(3, GROUPS[3])
            emit_rows(b, 2048, 3584)
            emit_fp8(4, GROUPS[4])
            emit_rows(b, 3584, 4096)
            encb = encbs[b]

            for g, sbs in enumerate(GROUPS):
                gsb = len(sbs)
                gw = gsb * SBLK
                g0 = sbs[0]
                encg8 = encg8s[g]

                # big projection (fp8 DoubleRow) + tanh, h-tile at a time
                tt_all = tpool.tile([128, HT * GW], F16, name="tanh")
                tanh_t = {}
                for h in range(HT):
                    proj = pp.tile([128, GW], F32, name="proj")
                    for j in range(NF8):
                        lhsT = We8_sb.rearrange(
                            "p (t m) -> p t m", t=ET)[
                            :, 2 * j:2 * j + 2, h * 128:(h + 1) * 128]
                        for i in range(gsb):
                            rhs = encg8[
                                :, j * 2 * gw + 2 * i * SBLK:
                                j * 2 * gw + 2 * (i + 1) * SBLK].rearrange(
                                "p (s two) -> p two s", two=2)
                            nc.tensor.matmul(
                                proj[:, i * SBLK:(i + 1) * SBLK], lhsT, rhs,
                                start=(j == 0),
                                stop=(j == NF8 - 1 and NF8 == 4),
                                perf_mode=mybir.MatmulPerfMode.DoubleRow,
                            )
                    for t in range(2 * NF8, ET):
                        lhsT = We16_sb[:, t * H + h * 128:t * H + (h + 1) * 128]
                        for i in range(gsb):
                            rhs = encb[:, t * S + g0 * SBLK + i * SBLK:
                                       t * S + g0 * SBLK + (i + 1) * SBLK]
                            nc.tensor.matmul(
                                proj[:, i * SBLK:(i + 1) * SBLK], lhsT, rhs,
                                start=False, stop=(t == ET - 1),
                            )
                    tt = tt_all[:, h * GW:h * GW + GW]
                    nc.scalar.activation(
                        tt[:, 0:gw], proj[:, 0:gw],
                        mybir.ActivationFunctionType.Tanh,
                        bias=hid_sb[:, h * BPC + b: h * BPC + b + 1],
                        scale=1.0 / SCALE_W,
                    )
                    tanh_t[h] = tt

                # logits (broadcast across partitions) + exp + Z accum
                wg = wpool.tile([128, GW], F16, name="wg")
                lg = pl.tile([128, GW], F32, name="lg", tag="lg")
                for h in range(HT):
                    for i in range(gsb):
                        nc.tensor.matmul(
                            lg[:, i * SBLK:(i + 1) * SBLK],
                            V_sb[:, h * 128:(h + 1) * 128],
                            tanh_t[h][:, i * SBLK:(i + 1) * SBLK],
                            start=(h == 0), stop=(h == HT - 1),
                        )
                nc.scalar.activation(
                    wg[:, 0:gw], lg[:, 0:gw],
                    mybir.ActivationFunctionType.Exp,
                    bias=shift_col,
                    accum_out=zslots[:, b * NGRP + g: b * NGRP + g + 1],
                )

                # flush previous group's deferred scalar accums: they sit
                # in the scalar FIFO after THIS group's exp, filling idle
                # time without delaying the tanh/exp chain
                for (dt, dcol, dgw) in pending:
                    jt2 = jpool.tile([128, GW], F16, name="junk2")
                    nc.scalar.activation(
                        jt2[:, 0:dgw], dt[:, 0:dgw],
                        mybir.ActivationFunctionType.Copy,
                        accum_out=cslots[:, dcol:dcol + 1])
                pending = []

                # context accumulation: sum_s w_s * enc16[e, s]
                for e in range(ET):
                    col = (b * ET + e) * NGRP + g
                    src = encb[:, e * S + g0 * SBLK: e * S + g0 * SBLK + gw]
                    if e < CTX_TT:
                        # DVE 2x multiply now, scalar accumulate next group
                        dt = dpool.tile([128, GW], F16, name="defer")
                        nc.vector.tensor_tensor(
                            dt[:, 0:gw], src, wg[:, 0:gw],
                            mybir.AluOpType.mult)
                        pending.append((dt, col, gw))
                    else:
                        jt = jpool.tile([128, GW], F16, name="junk")
                        nc.vector.scalar_tensor_tensor(
                            jt[:, 0:gw], src, 1.0, wg[:, 0:gw],
                            mybir.AluOpType.mult, mybir.AluOpType.mult,
                            accum_out=cslots[:, col:col + 1],
                        )

            # flush the last group's deferred accums
            for (dt, dcol, dgw) in pending:
                jt2 = jpool.tile([128, GW], F16, name="junk2")
                nc.scalar.activation(
                    jt2[:, 0:dgw], dt[:, 0:dgw],
                    mybir.ActivationFunctionType.Copy,
                    accum_out=cslots[:, dcol:dcol + 1])
            pending = []

            # finalize this batch: ctx = (sum_g ctx_partial) / Z
            nc.vector.tensor_reduce(
                ctx_red[:, b * ET:(b + 1) * ET],
                cslots[:, b * ET * NGRP:(b + 1) * ET * NGRP].rearrange(
                    "p (q s) -> p q s", s=NGRP),
                axis=mybir.AxisListType.X, op=mybir.AluOpType.add)
            nc.vector.tensor_reduce(
                zred[:, b:b + 1],
                zslots[:, b * NGRP:(b + 1) * NGRP].rearrange(
                    "p (q s) -> p q s", s=NGRP),
                axis=mybir.AxisListType.X, op=mybir.AluOpType.add)
            nc.vector.reciprocal(zrec[:, b:b + 1], zred[:, b:b + 1])
            nc.vector.tensor_scalar_mul(
                ctx_fin[:, b * ET:(b + 1) * ET],
                ctx_red[:, b * ET:(b + 1) * ET], zrec[:, b:b + 1])
            # out-DMA deferred to the end: an early trigger would block
            # the sync FIFO (and the next batch's fp8 stream) on this
            # batch's full context chain
            out_dmas.append(b)

        for b in out_dmas:
            nc.sync.dma_start(
                ctx_d[b].rearrange("(e p) -> p e", p=128),
                ctx_fin[:, b * ET:(b + 1) * ET])

    nc.compile()
    return nc


def kernel(hidden, encoder_outputs, W_attn, b_attn, v):
    global LAST_RESULTS
    hidden = np.asarray(hidden, dtype=np.float32)
    encoder_outputs = np.asarray(encoder_outputs, dtype=np.float32)
    W_attn = np.asarray(W_attn, dtype=np.float32)
    b_attn = np.asarray(b_attn, dtype=np.float32)
    v = np.asarray(v, dtype=np.float32)

    key = (NF8, NWARM, CTX_TT)
    if key not in _NC_CACHE:
        _NC_CACHE[key] = _build()
    nc = _NC_CACHE[key]

    We = W_attn[H:]
    We_rows = (We * SCALE_W).reshape(ET, 128, H).transpose(1, 0, 2).reshape(
        128, ET * H)
    We8 = np.ascontiguousarray(We_rows).astype(ml_dtypes.float8_e4m3)
    We16 = np.ascontiguousarray(We_rows).astype(np.float16)
    V128 = np.ascontiguousarray(np.broadcast_to(
        v.reshape(HT, 128, 1).transpose(1, 0, 2), (128, HT, 128)
    ).reshape(128, HT * 128).astype(np.float16))

    hid = (hidden @ W_attn[:H] + b_attn).astype(np.float32)     # (B, H)

    in_maps = []
    for c in range(NCORES):
        sl = slice(c * BPC, (c + 1) * BPC)
        encTc = np.ascontiguousarray(
            encoder_outputs[sl].transpose(0, 2, 1).astype(np.float16))
        # pair-element-interleaved fp8: encT8[b, j, p, s, i] = e4m3(encT[b, (2j+i)*128+p, s])
        e8 = encTc.astype(ml_dtypes.float8_e4m3).reshape(
            BPC, NPAIR, 2, 128, S).transpose(0, 1, 3, 4, 2)
        encT8c = np.ascontiguousarray(e8).reshape(BPC, NPAIR * 128, S * 2)
        consts = np.zeros((128, HT * BPC + 1), dtype=np.float32)
        consts[:, :HT * BPC] = hid[sl].reshape(
            BPC, HT, 128).transpose(2, 1, 0).reshape(128, HT * BPC)
        consts[:, HT * BPC] = -SHIFT
        in_maps.append({
            "encT": encTc, "encT8": encT8c, "We8": We8, "We16": We16,
            "V128": V128, "consts": consts,
        })

    res = run_bass_kernel_spmd(
        nc, in_maps, core_ids=list(range(NCORES)), trace=TRACE)
    LAST_RESULTS = res

    out = np.empty((B, 1, E), dtype=np.float32)
    for c in range(NCORES):
        out[c * BPC:(c + 1) * BPC, 0, :] = res.results[c]["ctx"]
    return out


# revision 18
# speedup vs baseline: 1.0188x; 1.0188x over previous
"""Bahdanau-style attention kernel for Trainium2 (8 NeuronCores, batch-parallel).

Computes, for B=16, S=4096, H=512:
    hid  = hidden @ W_attn[:H] + b_attn                       (B, H)
    en   = tanh(hid[:,None,:] + enc @ W_attn[H:])             (B, S, H)
    lg   = en @ v                                             (B, S, 1)
    w    = softmax(lg, axis=1)
    ctx  = w^T @ enc                                          (B, 1, 2H)

Sharding: data-parallel over batch, 2 batches per core. Per core:
  - enc streams from HBM twice: e-major f16 full rows (whole-batch SBUF
    residency, 8KB descriptors, feeds the context accumulation) and fp8
    e4m3 with e-tile pairs element-interleaved (group-streamed, feeds
    the double-pumped DoubleRow projection matmul, 2 e-tiles/matmul).
    We is pre-scaled by SCALE_W to dodge the e4m3 subnormal range; the
    1/SCALE_W is folded into tanh's scale.
  - hid projection is precomputed host-side (it is O(B*H*H), trivial
    next to the O(B*S*E*H) device work) and enters as tanh's bias.
  - logits use v replicated across 128 partitions as the f16 stationary
    so exp(logits) lands pre-broadcast; exp emits f16 weights (the
    constant shift keeps every weight inside f16 normal range) and its
    accum_out yields the softmax normalizer.
  - context = sum_s w_s * enc16[e, s]: mostly fused scalar_tensor_tensor
    on DVE; CTX_TT e-tiles per group instead run DVE 2x tensor_tensor
    with the accumulation on the scalar engine (load balancing).
No cross-core communication; output gathered on host.
"""

import os
import numpy as np
import ml_dtypes
from contextlib import ExitStack

import concourse.bacc as bacc
import concourse.tile as tile
from concourse import mybir
from concourse.bass_utils import run_bass_kernel_spmd

F32 = mybir.dt.float32
F16 = mybir.dt.float16
F8 = mybir.dt.float8e4

B, S, H = 16, 4096, 512
E = 2 * H                      # 1024 encoder feature dim
NCORES = 8
BPC = B // NCORES              # batches per core = 2
ET = E // 128                  # 8 e-tiles
HT = H // 128                  # 4 h-tiles
NPAIR = ET // 2                # 4 e-tile pairs for DoubleRow
SBLK = 512                     # s-block width
NSB = S // SBLK                # 8 s-blocks per batch
GROUP_SB = 2
GW = GROUP_SB * SBLK

SCALE_W = 32.0                 # We pre-scale for e4m3 range
SHIFT = 5.0                    # exp(logit - SHIFT): logits empirically in [-3.4, 3.4]

NF8 = int(os.environ.get("ATTN_NF8", "4"))
NWARM = int(os.environ.get("ATTN_NWARM", "18"))
CTX_TT = int(os.environ.get("ATTN_CTX_TT", "2"))

GROUPS = [[0, 1], [2, 3], [4, 5], [6], [7]]
NGRP = len(GROUPS)

TRACE = False          # set by test harness; harness-default off
LAST_RESULTS = None    # last BassKernelResults (for profiling in test.py)

_NC_CACHE = {}


def _build():
    nc = bacc.Bacc("TRN2", target_bir_lowering=False, debug=False)

    CW = HT * BPC + 1          # hid columns | -SHIFT
    encT = nc.dram_tensor("encT", [BPC, E, S], F16, kind="ExternalInput").ap()
    encT8 = nc.dram_tensor("encT8", [BPC, NPAIR * 128, S * 2], F8,
                           kind="ExternalInput").ap()
    We8_d = nc.dram_tensor("We8", [128, ET * H], F8, kind="ExternalInput").ap()
    We16_d = nc.dram_tensor("We16", [128, ET * H], F16, kind="ExternalInput").ap()
    V_d = nc.dram_tensor("V128", [128, HT * 128], F16, kind="ExternalInput").ap()
    cst_d = nc.dram_tensor("consts", [128, CW], F32, kind="ExternalInput").ap()
    ctx_d = nc.dram_tensor("ctx", [BPC, E], F32, kind="ExternalOutput").ap()

    with tile.TileContext(nc) as tc, ExitStack() as ctx:
        cpool = ctx.enter_context(tc.tile_pool(name="consts", bufs=1))
        fpool = ctx.enter_context(tc.tile_pool(name="encb", bufs=2))
        e8pool = ctx.enter_context(tc.tile_pool(name="enc8", bufs=4))
        tpool = ctx.enter_context(tc.tile_pool(name="tanh", bufs=2))
        wpool = ctx.enter_context(tc.tile_pool(name="wexp", bufs=3))
        jpool = ctx.enter_context(tc.tile_pool(name="junk", bufs=2))
        dpool = ctx.enter_context(tc.tile_pool(name="defer", bufs=5))
        spool = ctx.enter_context(tc.tile_pool(name="stats", bufs=1))
        pp = ctx.enter_context(tc.tile_pool(name="pproj", bufs=2, space="PSUM"))
        pl = ctx.enter_context(tc.tile_pool(name="plog", bufs=2, space="PSUM"))

        # ---- PE warm-up: dummy matmuls while DMAs land (HAM -> K=8/8) ----
        wlhs = cpool.tile([128, 128], F16)
        wrhs = cpool.tile([128, 256], F16)
        nc.vector.memset(wlhs[:], 0.0)
        nc.vector.memset(wrhs[:], 0.0)
        wps = pl.tile([128, GW], F32, name="lg", tag="lg")
        for _ in range(NWARM):
            nc.tensor.matmul(wps[:, 0:256], wlhs[:], wrhs[:], start=True, stop=True)

        # ---- constants ----
        cst_sb = cpool.tile([128, CW], F32)
        nc.scalar.dma_start(cst_sb[:], cst_d)
        We8_sb = cpool.tile([128, ET * H], F8)
        nc.scalar.dma_start(We8_sb[:], We8_d)
        V_sb = cpool.tile([128, HT * 128], F16)
        nc.scalar.dma_start(V_sb[:], V_d)
        We16_sb = None
        if NF8 < 4:
            We16_sb = cpool.tile([128, ET * H], F16)
            nc.scalar.dma_start(We16_sb[:], We16_d)
        hid_sb = cst_sb[:, 0:HT * BPC]            # bias col per (h, b)
        shift_col = cst_sb[:, HT * BPC:HT * BPC + 1]

        # ---- stats accumulators ----
        zslots = spool.tile([128, BPC * NGRP], F32)
        cslots = spool.tile([128, BPC * ET * NGRP], F32)
        ctx_red = spool.tile([128, BPC * ET], F32)
        zred = spool.tile([128, BPC], F32)
        zrec = spool.tile([128, BPC], F32)
        ctx_fin = spool.tile([128, BPC * ET], F32)

        out_dmas = []
        encbs = {}

        def emit_rows(bb, s0, s1):
            # f16 partial rows (2-4KB runs) for batch bb on the sync queue
            for t in range(ET):
                nc.sync.dma_start(
                    encbs[bb][:, t * S + s0:t * S + s1],
                    encT[bb].rearrange("(t p) s -> p t s", p=128)[
                        :, t, s0:s1],
                )

        for b in range(BPC):
            pending = []          # (tile, col, gw) scalar accums, one group late
            # ---- DMA emission pass: trigger order on the sync queue is
            # fp8 for groups 0..3, then the batch's f16 full rows, then
            # g4's fp8 (its pool-WAR semaphore delays it anyway) ----
            encg8s = []

            def emit_fp8(g, sbs):
                gsb = len(sbs)
                gw = gsb * SBLK
                g0 = sbs[0]
                encg8 = e8pool.tile([128, ET * GW], F8, name="encg8",
                                    tag="encg8")
                if b == 0 and g == 0:
                    # first data on the wire: pair 0, first s-block only
                    nc.sync.dma_start(
                        encg8[:, 0:2 * SBLK],
                        encT8[0].rearrange("(j p) s2 -> p j s2", p=128)[
                            :, 0, 0:2 * SBLK])
                    nc.sync.dma_start(
                        encg8[:, 2 * SBLK:2 * gw],
                        encT8[0].rearrange("(j p) s2 -> p j s2", p=128)[
                            :, 0, 2 * SBLK:2 * gw])
                    jchunks = [(1, 1), (2, 2)]
                else:
                    jchunks = [(0, 2), (2, 2)]
                for j0, nj in jchunks:
                    nc.sync.dma_start(
                        encg8[:, j0 * 2 * gw:(j0 + nj) * 2 * gw].rearrange(
                            "p (j s2) -> p j s2", j=nj),
                        encT8[b].rearrange("(j p) s2 -> p j s2", p=128)[
                            :, j0:j0 + nj,
                            g0 * SBLK * 2:g0 * SBLK * 2 + 2 * gw],
                    )
                encg8s.append(encg8)

            if b not in encbs:
                encbs[b] = fpool.tile([128, ET * S], F16, name="encb",
                                      tag="encb")
            emit_fp8(0, GROUPS[0])
            emit_fp8(1, GROUPS[1])
            emit_rows(b, 0, 2048)
            emit_fp8(2, GROUPS[2])
            emit_fp8(3, GROUPS[3])
            emit_rows(b, 2048, 3584)
            emit_fp8(4, GROUPS[4])
            emit_rows(b, 3584, 4096)
            encb = encbs[b]

            for g, sbs in enumerate(GROUPS):
                gsb = len(sbs)
                gw = gsb * SBLK
                g0 = sbs[0]
                encg8 = encg8s[g]

                # big projection (fp8 DoubleRow) + tanh, h-tile at a time
                tt_all = tpool.tile([128, HT * GW], F16, name="tanh")
                tanh_t = {}
                for h in range(HT):
                    proj = pp.tile([128, GW], F32, name="proj")
                    for j in range(NF8):
                        lhsT = We8_sb.rearrange(
                            "p (t m) -> p t m", t=ET)[
                            :, 2 * j:2 * j + 2, h * 128:(h + 1) * 128]
                        for i in range(gsb):
                            rhs = encg8[
                                :, j * 2 * gw + 2 * i * SBLK:
                                j * 2 * gw + 2 * (i + 1) * SBLK].rearrange(
                                "p (s two) -> p two s", two=2)
                            nc.tensor.matmul(
                                proj[:, i * SBLK:(i + 1) * SBLK], lhsT, rhs,
                                start=(j == 0),
                                stop=(j == NF8 - 1 and NF8 == 4),
                                perf_mode=mybir.MatmulPerfMode.DoubleRow,
                            )
                    for t in range(2 * NF8, ET):
                        lhsT = We16_sb[:, t * H + h * 128:t * H + (h + 1) * 128]
                        for i in range(gsb):
                            rhs = encb[:, t * S + g0 * SBLK + i * SBLK:
                                       t * S + g0 * SBLK + (i + 1) * SBLK]
                            nc.tensor.matmul(
                                proj[:, i * SBLK:(i + 1) * SBLK], lhsT, rhs,
                                start=False, stop=(t == ET - 1),
                            )
                    tt = tt_all[:, h * GW:h * GW + GW]
                    nc.scalar.activation(
                        tt[:, 0:gw], proj[:, 0:gw],
                        mybir.ActivationFunctionType.Tanh,
                        bias=hid_sb[:, h * BPC + b: h * BPC + b + 1],
                        scale=1.0 / SCALE_W,
                    )
                    tanh_t[h] = tt

                # logits (broadcast across partitions) + exp + Z accum
                wg = wpool.tile([128, GW], F16, name="wg")
                lg = pl.tile([128, GW], F32, name="lg", tag="lg")
                for h in range(HT):
                    for i in range(gsb):
                        nc.tensor.matmul(
                            lg[:, i * SBLK:(i + 1) * SBLK],
                            V_sb[:, h * 128:(h + 1) * 128],
                            tanh_t[h][:, i * SBLK:(i + 1) * SBLK],
                            start=(h == 0), stop=(h == HT - 1),
                        )
                nc.scalar.activation(
                    wg[:, 0:gw], lg[:, 0:gw],
                    mybir.ActivationFunctionType.Exp,
                    bias=shift_col,
                    accum_out=zslots[:, b * NGRP + g: b * NGRP + g + 1],
                )

                # flush deferred scalar accums: at most 2 per double
                # group / 1 per single group so the scalar FIFO backlog
                # never delays the next group's tanh (PE pp-buffer WAR)
                nflush = 2 if gsb > 1 else 1
                for (dt, dcol, dgw) in pending[:nflush]:
                    jt2 = jpool.tile([128, GW], F16, name="junk2")
                    nc.scalar.activation(
                        jt2[:, 0:dgw], dt[:, 0:dgw],
                        mybir.ActivationFunctionType.Copy,
                        accum_out=cslots[:, dcol:dcol + 1])
                pending = pending[nflush:]

                # context accumulation: sum_s w_s * enc16[e, s]
                for e in range(ET):
                    col = (b * ET + e) * NGRP + g
                    src = encb[:, e * S + g0 * SBLK: e * S + g0 * SBLK + gw]
                    if e < CTX_TT and gsb > 1:
                        # DVE 2x multiply now, scalar accumulate next group
                        dt = dpool.tile([128, GW], F16, name="defer")
                        nc.vector.tensor_tensor(
                            dt[:, 0:gw], src, wg[:, 0:gw],
                            mybir.AluOpType.mult)
                        pending.append((dt, col, gw))
                    else:
                        jt = jpool.tile([128, GW], F16, name="junk")
                        nc.vector.scalar_tensor_tensor(
                            jt[:, 0:gw], src, 1.0, wg[:, 0:gw],
                            mybir.AluOpType.mult, mybir.AluOpType.mult,
                            accum_out=cslots[:, col:col + 1],
                        )

            # flush the last group's deferred accums
            for (dt, dcol, dgw) in pending:
                jt2 = jpool.tile([128, GW], F16, name="junk2")
                nc.scalar.activation(
                    jt2[:, 0:dgw], dt[:, 0:dgw],
                    mybir.ActivationFunctionType.Copy,
                    accum_out=cslots[:, dcol:dcol + 1])
            pending = []

            # finalize this batch: ctx = (sum_g ctx_partial) / Z
            nc.vector.tensor_reduce(
                ctx_red[:, b * ET:(b + 1) * ET],
                cslots[:, b * ET * NGRP:(b + 1) * ET * NGRP].rearrange(
                    "p (q s) -> p q s", s=NGRP),
                axis=mybir.AxisListType.X, op=mybir.AluOpType.add)
            nc.vector.tensor_reduce(
                zred[:, b:b + 1],
                zslots[:, b * NGRP:(b + 1) * NGRP].rearrange(
                    "p (q s) -> p q s", s=NGRP),
                axis=mybir.AxisListType.X, op=mybir.AluOpType.add)
            nc.vector.reciprocal(zrec[:, b:b + 1], zred[:, b:b + 1])
            nc.vector.tensor_scalar_mul(
                ctx_fin[:, b * ET:(b + 1) * ET],
                ctx_red[:, b * ET:(b + 1) * ET], zrec[:, b:b + 1])
            # out-DMA deferred to the end: an early trigger would block
            # the sync FIFO (and the next batch's fp8 stream) on this
            # batch's full context chain
            out_dmas.append(b)

        for b in out_dmas:
            nc.sync.dma_start(
                ctx_d[b].rearrange("(e p) -> p e", p=128),
                ctx_fin[:, b * ET:(b + 1) * ET])

    nc.compile()
    return nc


def kernel(hidden, encoder_outputs, W_attn, b_attn, v):
    global LAST_RESULTS
    hidden = np.asarray(hidden, dtype=np.float32)
    encoder_outputs = np.asarray(encoder_outputs, dtype=np.float32)
    W_attn = np.asarray(W_attn, dtype=np.float32)
    b_attn = np.asarray(b_attn, dtype=np.float32)
    v = np.asarray(v, dtype=np.float32)

    key = (NF8, NWARM, CTX_TT)
    if key not in _NC_CACHE:
        _NC_CACHE[key] = _build()
    nc = _NC_CACHE[key]

    We = W_attn[H:]
    We_rows = (We * SCALE_W).reshape(ET, 128, H).transpose(1, 0, 2).reshape(
        128, ET * H)
    We8 = np.ascontiguousarray(We_rows).astype(ml_dtypes.float8_e4m3)
    We16 = np.ascontiguousarray(We_rows).astype(np.float16)
    V128 = np.ascontiguousarray(np.broadcast_to(
        v.reshape(HT, 128, 1).transpose(1, 0, 2), (128, HT, 128)
    ).reshape(128, HT * 128).astype(np.float16))

    hid = (hidden @ W_attn[:H] + b_attn).astype(np.float32)     # (B, H)

    in_maps = []
    for c in range(NCORES):
        sl = slice(c * BPC, (c + 1) * BPC)
        encTc = np.ascontiguousarray(
            encoder_outputs[sl].transpose(0, 2, 1).astype(np.float16))
        # pair-element-interleaved fp8: encT8[b, j, p, s, i] = e4m3(encT[b, (2j+i)*128+p, s])
        e8 = encTc.astype(ml_dtypes.float8_e4m3).reshape(
            BPC, NPAIR, 2, 128, S).transpose(0, 1, 3, 4, 2)
        encT8c = np.ascontiguousarray(e8).reshape(BPC, NPAIR * 128, S * 2)
        consts = np.zeros((128, HT * BPC + 1), dtype=np.float32)
        consts[:, :HT * BPC] = hid[sl].reshape(
            BPC, HT, 128).transpose(2, 1, 0).reshape(128, HT * BPC)
        consts[:, HT * BPC] = -SHIFT
        in_maps.append({
            "encT": encTc, "encT8": encT8c, "We8": We8, "We16": We16,
            "V128": V128, "consts": consts,
        })

    res = run_bass_kernel_spmd(
        nc, in_maps, core_ids=list(range(NCORES)), trace=TRACE)
    LAST_RESULTS = res

    out = np.empty((B, 1, E), dtype=np.float32)
    for c in range(NCORES):
        out[c * BPC:(c + 1) * BPC, 0, :] = res.results[c]["ctx"]
    return out


# revision 19
# speedup vs baseline: 1.0301x; 1.0111x over previous
"""Bahdanau-style attention kernel for Trainium2 (8 NeuronCores, batch-parallel).

Computes, for B=16, S=4096, H=512:
    hid  = hidden @ W_attn[:H] + b_attn                       (B, H)
    en   = tanh(hid[:,None,:] + enc @ W_attn[H:])             (B, S, H)
    lg   = en @ v                                             (B, S, 1)
    w    = softmax(lg, axis=1)
    ctx  = w^T @ enc                                          (B, 1, 2H)

Sharding: data-parallel over batch, 2 batches per core. Per core:
  - enc streams from HBM twice: e-major f16 full rows (whole-batch SBUF
    residency, 8KB descriptors, feeds the context accumulation) and fp8
    e4m3 with e-tile pairs element-interleaved (group-streamed, feeds
    the double-pumped DoubleRow projection matmul, 2 e-tiles/matmul).
    We is pre-scaled by SCALE_W to dodge the e4m3 subnormal range; the
    1/SCALE_W is folded into tanh's scale.
  - hid projection is precomputed host-side (it is O(B*H*H), trivial
    next to the O(B*S*E*H) device work) and enters as tanh's bias.
  - logits use v replicated across 128 partitions as the f16 stationary
    so exp(logits) lands pre-broadcast; exp emits f16 weights (the
    constant shift keeps every weight inside f16 normal range) and its
    accum_out yields the softmax normalizer.
  - context = sum_s w_s * enc16[e, s]: mostly fused scalar_tensor_tensor
    on DVE; CTX_TT e-tiles per group instead run DVE 2x tensor_tensor
    with the accumulation on the scalar engine (load balancing).
No cross-core communication; output gathered on host.
"""

import os
import numpy as np
import ml_dtypes
from contextlib import ExitStack

import concourse.bacc as bacc
import concourse.tile as tile
from concourse import mybir
from concourse.bass_utils import run_bass_kernel_spmd

F32 = mybir.dt.float32
F16 = mybir.dt.float16
F8 = mybir.dt.float8e4

B, S, H = 16, 4096, 512
E = 2 * H                      # 1024 encoder feature dim
NCORES = 8
BPC = B // NCORES              # batches per core = 2
ET = E // 128                  # 8 e-tiles
HT = H // 128                  # 4 h-tiles
NPAIR = ET // 2                # 4 e-tile pairs for DoubleRow
SBLK = 512                     # s-block width
NSB = S // SBLK                # 8 s-blocks per batch
GROUP_SB = 2
GW = GROUP_SB * SBLK

SCALE_W = 32.0                 # We pre-scale for e4m3 range
SHIFT = 5.0                    # exp(logit - SHIFT): logits empirically in [-3.4, 3.4]

NF8 = int(os.environ.get("ATTN_NF8", "4"))
NWARM = int(os.environ.get("ATTN_NWARM", "18"))
CTX_TT = int(os.environ.get("ATTN_CTX_TT", "2"))

GROUPS = [[0, 1], [2, 3], [4, 5], [6], [7]]
NGRP = len(GROUPS)

TRACE = False          # set by test harness; harness-default off
LAST_RESULTS = None    # last BassKernelResults (for profiling in test.py)

_NC_CACHE = {}


def _build():
    nc = bacc.Bacc("TRN2", target_bir_lowering=False, debug=False)

    CW = HT * BPC + 1          # hid columns | -SHIFT
    encT = nc.dram_tensor("encT", [BPC, E, S], F16, kind="ExternalInput").ap()
    encT8 = nc.dram_tensor("encT8", [BPC, NPAIR * 128, S * 2], F8,
                           kind="ExternalInput").ap()
    We8_d = nc.dram_tensor("We8", [128, ET * H], F8, kind="ExternalInput").ap()
    We16_d = nc.dram_tensor("We16", [128, ET * H], F16, kind="ExternalInput").ap()
    V_d = nc.dram_tensor("V128", [128, HT * 128], F16, kind="ExternalInput").ap()
    cst_d = nc.dram_tensor("consts", [128, CW], F32, kind="ExternalInput").ap()
    ctx_d = nc.dram_tensor("ctx", [BPC, E], F32, kind="ExternalOutput").ap()

    with tile.TileContext(nc) as tc, ExitStack() as ctx:
        cpool = ctx.enter_context(tc.tile_pool(name="consts", bufs=1))
        fpool = ctx.enter_context(tc.tile_pool(name="encb", bufs=2))
        e8pool = ctx.enter_context(tc.tile_pool(name="enc8", bufs=4))
        tpool = ctx.enter_context(tc.tile_pool(name="tanh", bufs=2))
        wpool = ctx.enter_context(tc.tile_pool(name="wexp", bufs=3))
        jpool = ctx.enter_context(tc.tile_pool(name="junk", bufs=2))
        dpool = ctx.enter_context(tc.tile_pool(name="defer", bufs=5))
        spool = ctx.enter_context(tc.tile_pool(name="stats", bufs=1))
        pp = ctx.enter_context(tc.tile_pool(name="pproj", bufs=2, space="PSUM"))
        pl = ctx.enter_context(tc.tile_pool(name="plog", bufs=2, space="PSUM"))

        # ---- PE warm-up: dummy matmuls while DMAs land (HAM -> K=8/8) ----
        wlhs = cpool.tile([128, 128], F16)
        wrhs = cpool.tile([128, 256], F16)
        nc.vector.memset(wlhs[:], 0.0)
        nc.vector.memset(wrhs[:], 0.0)
        wps = pl.tile([128, GW], F32, name="lg", tag="lg")
        for _ in range(NWARM):
            nc.tensor.matmul(wps[:, 0:256], wlhs[:], wrhs[:], start=True, stop=True)

        # ---- constants ----
        cst_sb = cpool.tile([128, CW], F32)
        nc.scalar.dma_start(cst_sb[:], cst_d)
        We8_sb = cpool.tile([128, ET * H], F8)
        nc.scalar.dma_start(We8_sb[:], We8_d)
        V_sb = cpool.tile([128, HT * 128], F16)
        nc.scalar.dma_start(V_sb[:], V_d)
        We16_sb = None
        if NF8 < 4:
            We16_sb = cpool.tile([128, ET * H], F16)
            nc.scalar.dma_start(We16_sb[:], We16_d)
        hid_sb = cst_sb[:, 0:HT * BPC]            # bias col per (h, b)
        shift_col = cst_sb[:, HT * BPC:HT * BPC + 1]

        # ---- stats accumulators ----
        zslots = spool.tile([128, BPC * NGRP], F32)
        cslots = spool.tile([128, BPC * ET * NGRP], F32)
        ctx_red = spool.tile([128, BPC * ET], F32)
        zred = spool.tile([128, BPC], F32)
        zrec = spool.tile([128, BPC], F32)
        ctx_fin = spool.tile([128, BPC * ET], F32)

        out_dmas = []
        encbs = {}

        def emit_rows(bb, s0, s1):
            # f16 partial rows (2-4KB runs) for batch bb on the sync queue
            for t in range(ET):
                nc.sync.dma_start(
                    encbs[bb][:, t * S + s0:t * S + s1],
                    encT[bb].rearrange("(t p) s -> p t s", p=128)[
                        :, t, s0:s1],
                )

        for b in range(BPC):
            pending = []          # (tile, col, gw) scalar accums, one group late
            # ---- DMA emission pass: trigger order on the sync queue is
            # fp8 for groups 0..3, then the batch's f16 full rows, then
            # g4's fp8 (its pool-WAR semaphore delays it anyway) ----
            encg8s = []

            def emit_fp8(g, sbs):
                gsb = len(sbs)
                gw = gsb * SBLK
                g0 = sbs[0]
                encg8 = e8pool.tile([128, ET * GW], F8, name="encg8",
                                    tag="encg8")
                if b == 0 and g == 0:
                    # first data on the wire: pair 0, first s-block only
                    nc.sync.dma_start(
                        encg8[:, 0:2 * SBLK],
                        encT8[0].rearrange("(j p) s2 -> p j s2", p=128)[
                            :, 0, 0:2 * SBLK])
                    nc.sync.dma_start(
                        encg8[:, 2 * SBLK:2 * gw],
                        encT8[0].rearrange("(j p) s2 -> p j s2", p=128)[
                            :, 0, 2 * SBLK:2 * gw])
                    jchunks = [(1, 1), (2, 2)]
                else:
                    jchunks = [(0, 2), (2, 2)]
                for j0, nj in jchunks:
                    nc.sync.dma_start(
                        encg8[:, j0 * 2 * gw:(j0 + nj) * 2 * gw].rearrange(
                            "p (j s2) -> p j s2", j=nj),
                        encT8[b].rearrange("(j p) s2 -> p j s2", p=128)[
                            :, j0:j0 + nj,
                            g0 * SBLK * 2:g0 * SBLK * 2 + 2 * gw],
                    )
                encg8s.append(encg8)

            if b not in encbs:
                encbs[b] = fpool.tile([128, ET * S], F16, name="encb",
                                      tag="encb")
            emit_fp8(0, GROUPS[0])
            emit_fp8(1, GROUPS[1])
            emit_rows(b, 0, 2048)
            emit_fp8(2, GROUPS[2])
            emit_fp8(3, GROUPS[3])
            emit_rows(b, 2048, 3584)
            emit_fp8(4, GROUPS[4])
            emit_rows(b, 3584, 4096)
            encb = encbs[b]

            for g, sbs in enumerate(GROUPS):
                gsb = len(sbs)
                gw = gsb * SBLK
                g0 = sbs[0]
                encg8 = encg8s[g]

                # big projection (fp8 DoubleRow) + tanh, h-tile at a time
                tt_all = tpool.tile([128, HT * GW], F16, name="tanh")
                tanh_t = {}
                for h in range(HT):
                    proj = pp.tile([128, GW], F32, name="proj")
                    for j in range(NF8):
                        lhsT = We8_sb.rearrange(
                            "p (t m) -> p t m", t=ET)[
                            :, 2 * j:2 * j + 2, h * 128:(h + 1) * 128]
                        for i in range(gsb):
                            rhs = encg8[
                                :, j * 2 * gw + 2 * i * SBLK:
                                j * 2 * gw + 2 * (i + 1) * SBLK].rearrange(
                                "p (s two) -> p two s", two=2)
                            nc.tensor.matmul(
                                proj[:, i * SBLK:(i + 1) * SBLK], lhsT, rhs,
                                start=(j == 0),
                                stop=(j == NF8 - 1 and NF8 == 4),
                                perf_mode=mybir.MatmulPerfMode.DoubleRow,
                            )
                    for t in range(2 * NF8, ET):
                        lhsT = We16_sb[:, t * H + h * 128:t * H + (h + 1) * 128]
                        for i in range(gsb):
                            rhs = encb[:, t * S + g0 * SBLK + i * SBLK:
                                       t * S + g0 * SBLK + (i + 1) * SBLK]
                            nc.tensor.matmul(
                                proj[:, i * SBLK:(i + 1) * SBLK], lhsT, rhs,
                                start=False, stop=(t == ET - 1),
                            )
                    tt = tt_all[:, h * GW:h * GW + GW]
                    nc.scalar.activation(
                        tt[:, 0:gw], proj[:, 0:gw],
                        mybir.ActivationFunctionType.Tanh,
                        bias=hid_sb[:, h * BPC + b: h * BPC + b + 1],
                        scale=1.0 / SCALE_W,
                    )
                    tanh_t[h] = tt

                # logits (broadcast across partitions) + exp + Z accum
                wg = wpool.tile([128, GW], F16, name="wg")
                lg = pl.tile([128, GW], F32, name="lg", tag="lg")
                for h in range(HT):
                    for i in range(gsb):
                        nc.tensor.matmul(
                            lg[:, i * SBLK:(i + 1) * SBLK],
                            V_sb[:, h * 128:(h + 1) * 128],
                            tanh_t[h][:, i * SBLK:(i + 1) * SBLK],
                            start=(h == 0), stop=(h == HT - 1),
                        )
                nc.scalar.activation(
                    wg[:, 0:gw], lg[:, 0:gw],
                    mybir.ActivationFunctionType.Exp,
                    bias=shift_col,
                    accum_out=zslots[:, b * NGRP + g: b * NGRP + g + 1],
                )

                # flush deferred scalar accums: at most 2 per double
                # group / 1 per single group so the scalar FIFO backlog
                # never delays the next group's tanh (PE pp-buffer WAR)
                nflush = 2 if gsb > 1 else 1
                with tc.high_priority(offset=-80):
                    # negative offset = LOWER priority: the scheduler must
                    # not slot these filler COPYs ahead of the next group's
                    # tanh in the strict-FIFO scalar queue
                    for (dt, dcol, dgw) in pending[:nflush]:
                        jt2 = jpool.tile([128, GW], F16, name="junk2")
                        nc.scalar.activation(
                            jt2[:, 0:dgw], dt[:, 0:dgw],
                            mybir.ActivationFunctionType.Copy,
                            accum_out=cslots[:, dcol:dcol + 1])
                pending = pending[nflush:]

                # context accumulation: sum_s w_s * enc16[e, s]
                for e in range(ET):
                    col = (b * ET + e) * NGRP + g
                    src = encb[:, e * S + g0 * SBLK: e * S + g0 * SBLK + gw]
                    if e < CTX_TT and gsb > 1:
                        # DVE 2x multiply now, scalar accumulate next group
                        dt = dpool.tile([128, GW], F16, name="defer")
                        nc.vector.tensor_tensor(
                            dt[:, 0:gw], src, wg[:, 0:gw],
                            mybir.AluOpType.mult)
                        pending.append((dt, col, gw))
                    else:
                        jt = jpool.tile([128, GW], F16, name="junk")
                        nc.vector.scalar_tensor_tensor(
                            jt[:, 0:gw], src, 1.0, wg[:, 0:gw],
                            mybir.AluOpType.mult, mybir.AluOpType.mult,
                            accum_out=cslots[:, col:col + 1],
                        )

            # flush the last group's deferred accums
            for (dt, dcol, dgw) in pending:
                jt2 = jpool.tile([128, GW], F16, name="junk2")
                nc.scalar.activation(
                    jt2[:, 0:dgw], dt[:, 0:dgw],
                    mybir.ActivationFunctionType.Copy,
                    accum_out=cslots[:, dcol:dcol + 1])
            pending = []

            # finalize this batch: ctx = (sum_g ctx_partial) / Z
            nc.vector.tensor_reduce(
                ctx_red[:, b * ET:(b + 1) * ET],
                cslots[:, b * ET * NGRP:(b + 1) * ET * NGRP].rearrange(
                    "p (q s) -> p q s", s=NGRP),
                axis=mybir.AxisListType.X, op=mybir.AluOpType.add)
            nc.vector.tensor_reduce(
                zred[:, b:b + 1],
                zslots[:, b * NGRP:(b + 1) * NGRP].rearrange(
                    "p (q s) -> p q s", s=NGRP),
                axis=mybir.AxisListType.X, op=mybir.AluOpType.add)
            nc.vector.reciprocal(zrec[:, b:b + 1], zred[:, b:b + 1])
            nc.vector.tensor_scalar_mul(
                ctx_fin[:, b * ET:(b + 1) * ET],
                ctx_red[:, b * ET:(b + 1) * ET], zrec[:, b:b + 1])
            # out-DMA deferred to the end: an early trigger would block
            # the sync FIFO (and the next batch's fp8 stream) on this
            # batch's full context chain
            out_dmas.append(b)

        for b in out_dmas:
            nc.sync.dma_start(
                ctx_d[b].rearrange("(e p) -> p e", p=128),
                ctx_fin[:, b * ET:(b + 1) * ET])

    nc.compile()
    return nc


def kernel(hidden, encoder_outputs, W_attn, b_attn, v):
    global LAST_RESULTS
    hidden = np.asarray(hidden, dtype=np.float32)
    encoder_outputs = np.asarray(encoder_outputs, dtype=np.float32)
    W_attn = np.asarray(W_attn, dtype=np.float32)
    b_attn = np.asarray(b_attn, dtype=np.float32)
    v = np.asarray(v, dtype=np.float32)

    key = (NF8, NWARM, CTX_TT)
    if key not in _NC_CACHE:
        _NC_CACHE[key] = _build()
    nc = _NC_CACHE[key]

    We = W_attn[H:]
    We_rows = (We * SCALE_W).reshape(ET, 128, H).transpose(1, 0, 2).reshape(
        128, ET * H)
    We8 = np.ascontiguousarray(We_rows).astype(ml_dtypes.float8_e4m3)
    We16 = np.ascontiguousarray(We_rows).astype(np.float16)
    V128 = np.ascontiguousarray(np.broadcast_to(
        v.reshape(HT, 128, 1).transpose(1, 0, 2), (128, HT, 128)
    ).reshape(128, HT * 128).astype(np.float16))

    hid = (hidden @ W_attn[:H] + b_attn).astype(np.float32)     # (B, H)

    in_maps = []
    for c in range(NCORES):
        sl = slice(c * BPC, (c + 1) * BPC)
        encTc = np.ascontiguousarray(
            encoder_outputs[sl].transpose(0, 2, 1).astype(np.float16))
        # pair-element-interleaved fp8: encT8[b, j, p, s, i] = e4m3(encT[b, (2j+i)*128+p, s])
        e8 = encTc.astype(ml_dtypes.float8_e4m3).reshape(
            BPC, NPAIR, 2, 128, S).transpose(0, 1, 3, 4, 2)
        encT8c = np.ascontiguousarray(e8).reshape(BPC, NPAIR * 128, S * 2)
        consts = np.zeros((128, HT * BPC + 1), dtype=np.float32)
        consts[:, :HT * BPC] = hid[sl].reshape(
            BPC, HT, 128).transpose(2, 1, 0).reshape(128, HT * BPC)
        consts[:, HT * BPC] = -SHIFT
        in_maps.append({
            "encT": encTc, "encT8": encT8c, "We8": We8, "We16": We16,
            "V128": V128, "consts": consts,
        })

    res = run_bass_kernel_spmd(
        nc, in_maps, core_ids=list(range(NCORES)), trace=TRACE)
    LAST_RESULTS = res

    out = np.empty((B, 1, E), dtype=np.float32)
    for c in range(NCORES):
        out[c * BPC:(c + 1) * BPC, 0, :] = res.results[c]["ctx"]
    return out
